# revision 67
# baseline (speedup 1.0000x reference)
"""Trainium2 Bass kernel for nn_DynamicConvolution.

Reference computation (per batch b, T=4096 timesteps, C=512 channels):
    h  = x @ w_in.T + b_in                    # (T, 2C)
    xg = h[:, :C] * sigmoid(h[:, C:])         # GLU -> (T, C)
    w  = softmax((xg @ w_wt.T + b_wt).reshape(T, H, K), axis=-1)
    out[c, t] = sum_k xg[t+k-3, c] * w[t, h(c), k]    # depthwise dynamic conv
    y  = (out + conv_bias) @ w_out.T + b_out

Sharding: data-parallel over batch B=8 -> one batch element per NeuronCore.
Each core runs an identical program on its slice; no collectives.

Per-core dataflow (all matmuls bf16, fp32 accumulation):
  - x is PE-transposed to xT (C-major) to feed mm1 (contraction over C).
  - mm1 produces h token-major; GLU on ACT+DVE -> xg (token-major, bf16).
  - xg is PE-transposed to xgT for the weight-projection matmul.
  - softmax over K on DVE/ACT -> wsm stored [p, j, m] (token-major).
  - The dynamic conv is computed as a banded matmul per (h, time-tile):
    out_h = xg_slab.T @ D, where D[t', t] = w[h, t'-t+3, t] is a 7-diagonal
    band matrix. D is materialized with one gpsimd local_scatter per time
    tile from a pre-shifted copy of the softmax weights (data_all); the
    per-partition scatter indices are host-precomputed constants.
  - Cross-tile band halo is handled by a second tiny matmul (N=4 columns)
    accumulating into the next tile's PSUM.
  - mm_out contracts C (conv output is C-major already) -> y.
"""

import os
import sys

import numpy as np

for _p in ("/opt/trn_rl_repo", os.path.expanduser("~/.axon_site/_ro/trn_rl_repo")):
    if os.path.isdir(_p) and _p not in sys.path:
        sys.path.insert(0, _p)

import concourse.bacc as bacc
import concourse.bass as bass
import concourse.mybir as mybir
import concourse.tile as tile
from concourse.bass_utils import run_bass_kernel_spmd

try:
    import ml_dtypes

    BF16 = np.dtype(ml_dtypes.bfloat16)
except ImportError:  # pragma: no cover
    BF16 = None

T, B, C = 4096, 8, 512
H, K = 8, 7
PAD_L = K // 2
C2 = 2 * C
HK = H * K  # 56
P = 128

F32 = mybir.dt.float32
BF = mybir.dt.bfloat16
I16 = mybir.dt.int16

# Dt tile layout: per h a 136-wide block holding the 134 band columns of one
# 128-timestep tile (columns j <-> t = t0 + j - 3).
MAIN_W = 136
DT_W = H * MAIN_W  # 1088


def ts(i, size):
    return slice(i * size, (i + 1) * size)


def host_scatter_idxs():
    """Scatter index table: data element (p, i, h) -> column of the Dt tile.

    data[p, i*8+h] = wsm[t0 + p + i - 3, 7h + 6 - i]; its band column is
    j = p + i (column j of block h covers output time t0 + j - 3).
    """
    p = np.arange(P)[:, None, None]
    i = np.arange(K)[None, :, None]
    h = np.arange(H)[None, None, :]
    idx = MAIN_W * h + p + i
    return np.ascontiguousarray(idx.reshape(P, K * H).astype(np.int16))


def build_nc(t_len=T, with_bias_in=False, with_bias_wt=False, with_bias_out=False,
             with_conv_bias=False, dbg=False):
    """Build the single-core Bass program (shared by all 8 cores)."""
    NT = t_len // P  # time tiles of 128
    NT4 = t_len // 512  # time tiles of 512 used by mm1/mm_out

    nc = bacc.Bacc()

    from contextlib import ExitStack
    _psum_stack = ExitStack()

    def ctx_enter(cm):
        return _psum_stack.enter_context(cm)

    def ctx_exit():
        _psum_stack.close()

    x_d = nc.declare_dram_parameter("xT", [C, t_len], F32, isOutput=False)
    w_inT_d = nc.declare_dram_parameter("w_inT", [P, 4, C2], BF, isOutput=False)
    w_wtT_d = nc.declare_dram_parameter("w_wtT", [P, 4, HK], BF, isOutput=False)
    w_outT_d = nc.declare_dram_parameter("w_outT", [P, 4, C], BF, isOutput=False)
    idxs_d = nc.declare_dram_parameter("idxs", [P, HK], I16, isOutput=False)
    ident16_d = nc.declare_dram_parameter("ident16", [P, P], BF, isOutput=False)
    sones8_d = nc.declare_dram_parameter("sones8", [HK, H], BF, isOutput=False)
    sones56_d = nc.declare_dram_parameter("sones56", [H, HK], BF, isOutput=False)
    if with_bias_in:
        b_in_d = nc.declare_dram_parameter("b_in", [C2], F32, isOutput=False)
    if with_bias_wt:
        b_wt_d = nc.declare_dram_parameter("b_wt", [HK], F32, isOutput=False)
    if with_bias_out:
        b_out_d = nc.declare_dram_parameter("b_out", [C], F32, isOutput=False)
    if with_conv_bias:
        cb4_d = nc.declare_dram_parameter("cb4", [P, 4], F32, isOutput=False)
    y_d = nc.declare_dram_parameter("y", [t_len, C], F32, isOutput=True)
    if dbg:
        NTd = t_len // P
        xg_dbg = nc.declare_dram_parameter("xg_dbg", [P, NTd, C], BF, isOutput=True)
        xgT_dbg = nc.declare_dram_parameter("xgT_dbg", [P, 4, t_len], BF, isOutput=True)
        wsm_dbg = nc.declare_dram_parameter("wsm_dbg", [P, K, NTd, H], BF, isOutput=True)
        data_dbg = nc.declare_dram_parameter("data_dbg", [P, NTd, HK], BF, isOutput=True)
        conv_dbg = nc.declare_dram_parameter("conv_dbg", [P, 4, t_len], BF, isOutput=True)
        dt_dbg = nc.declare_dram_parameter("dt_dbg", [P, NTd, DT_W], BF, isOutput=True)

    with tile.TileContext(nc) as tc:
        with (
            tc.tile_pool(name="const", bufs=1) as const,
            tc.tile_pool(name="big", bufs=1) as big,
            tc.tile_pool(name="xin", bufs=3) as xin,
            tc.tile_pool(name="work", bufs=3) as work,
            tc.tile_pool(name="dtp", bufs=3) as dtp,
            tc.tile_pool(name="outp", bufs=3) as outp,
        ):
            # ---- constants ----
            sb_winT = const.tile([P, 4, C2], BF)
            nc.sync.dma_start(sb_winT[:], w_inT_d[:])
            sb_wwtT = const.tile([P, 4, HK], BF)
            nc.sync.dma_start(sb_wwtT[:], w_wtT_d[:])
            sb_woutT = const.tile([P, 4, C], BF)
            nc.sync.dma_start(sb_woutT[:], w_outT_d[:])
            sb_idxs = const.tile([P, HK], I16)
            nc.sync.dma_start(sb_idxs[:], idxs_d[:])
            sb_id16 = const.tile([P, P], BF)
            nc.sync.dma_start(sb_id16[:], ident16_d[:])
            sb_so8 = const.tile([HK, H], BF)
            nc.sync.dma_start(sb_so8[:], sones8_d[:])
            sb_so56 = const.tile([H, HK], BF)
            nc.sync.dma_start(sb_so56[:], sones56_d[:])
            if with_bias_in:
                sb_bin = const.tile([P, C2], F32)
                nc.sync.dma_start(sb_bin[:], b_in_d[None, :].to_broadcast((P, C2)))
            if with_bias_wt:
                sb_bwt = const.tile([HK, 1], F32)
                nc.sync.dma_start(sb_bwt[:], b_wt_d[:, None])
            if with_bias_out:
                sb_bout = const.tile([P, C], F32)
                nc.sync.dma_start(sb_bout[:], b_out_d[None, :].to_broadcast((P, C)))
            if with_conv_bias:
                sb_cb4 = const.tile([P, 4], F32)
                nc.sync.dma_start(sb_cb4[:], cb4_d[:])

            # ---- persistent activations ----
            xT = big.tile([P, 4, t_len], BF)       # [c%128, c//128, t]
            xg = big.tile([P, NT, C], BF)          # [t%128, t//128, c]
            xgT = big.tile([P, 4, t_len], BF)      # [c%128, c//128, t]
            conv = big.tile([P, 4, t_len], BF)     # [c%128, c//128, t]
            wsm3 = big.tile([P, K, NT, H], BF)     # [t%128, k, t//128, h]
            data_tmp = big.tile([P, K, NT, H], BF)
            data_all = big.tile([P, NT, HK], BF)

            # ======== pass 1a: xT load (host pre-transposed), f32->bf16
            # cast by the SWDGE dma ========
            ps_mm1 = ctx_enter(tc.tile_pool(name="ps_mm1", bufs=2,
                                            space=bass.MemorySpace.PSUM))
            ps_tr = ctx_enter(tc.tile_pool(name="ps_tr", bufs=2,
                                           space=bass.MemorySpace.PSUM))
            for m4 in range(NT4):
                for q in range(4):
                    nc.gpsimd.dma_start(xT[:, q, ts(m4, 512)],
                                        x_d[ts(q, P), ts(m4, 512)])

            nc.gpsimd.memset(data_tmp[:], 0.0)

            def pass1b_tile(m):
                ps_a = ps_mm1.tile([P, C], F32, tag="ps_a")
                ps_g = ps_mm1.tile([P, C], F32, tag="ps_g")
                for q in range(4):
                    lhs = xT[:, q, ts(m, P)]
                    nc.tensor.matmul(ps_a[:], lhs, sb_winT[:, q, 0:C],
                                     start=(q == 0), stop=(q == 3))
                    nc.tensor.matmul(ps_g[:], lhs, sb_winT[:, q, C:C2],
                                     start=(q == 0), stop=(q == 3))
                sig = work.tile([P, C], F32, tag="sig")
                if with_bias_in:
                    tmp_g = work.tile([P, C], F32, tag="tmp_g")
                    nc.vector.tensor_add(tmp_g[:], ps_g[:], sb_bin[:, C:C2])
                    nc.scalar.activation(sig[:], tmp_g[:],
                                         mybir.ActivationFunctionType.Sigmoid)
                    tmp_a = work.tile([P, C], F32, tag="tmp_a")
                    nc.vector.tensor_add(tmp_a[:], ps_a[:], sb_bin[:, 0:C])
                    nc.vector.tensor_mul(xg[:, m, :], tmp_a[:], sig[:])
                else:
                    nc.scalar.activation(sig[:], ps_g[:],
                                         mybir.ActivationFunctionType.Sigmoid)
                    nc.vector.tensor_mul(xg[:, m, :], ps_a[:], sig[:])
                # xg -> xgT via PE transpose
                pxgT = ps_tr.tile([P, 4, P], BF, tag="pxgT")
                for q in range(4):
                    nc.tensor.transpose(pxgT[:, q, :], xg[:, m, ts(q, P)], sb_id16[:])
                nc.scalar.copy(xgT[:, :, ts(m, P)], pxgT[:])

            def pass1c_tile(n):
                # dynamic weights + softmax in the C-major [hk, t] domain:
                # exp (logits are bounded, no max-subtract), K-sums and 1/s
                # broadcast via tiny PE matmuls, PE-transpose to token-major
                pw2 = ps_wl.tile([HK, 512], F32, tag="w1")
                for q in range(4):
                    nc.tensor.matmul(pw2[:], sb_wwtT[:, q, :],
                                     xgT[:, q, ts(n, 512)],
                                     start=(q == 0), stop=(q == 3))
                e2 = work.tile([HK, 512], BF, tag="e2")
                if with_bias_wt:
                    nc.scalar.activation(e2[:], pw2[:],
                                         mybir.ActivationFunctionType.Exp,
                                         bias=sb_bwt[:])
                else:
                    nc.scalar.activation(e2[:], pw2[:],
                                         mybir.ActivationFunctionType.Exp)
                ps_s = ps_ss.tile([H, 512], F32, tag="ps_s")
                nc.tensor.matmul(ps_s[:], sb_so8[:], e2[:], start=True, stop=True)
                r8f = work.tile([H, 512], F32, tag="r8f")
                nc.vector.reciprocal_approx_fast(r8f[:], ps_s[:])
                r8 = work.tile([H, 512], BF, tag="r8")
                with nc.allow_low_precision(reason="softmax 1/s in bf16 is fine"):
                    nc.vector.tensor_copy(r8[:], r8f[:])
                ps_r = ps_ss.tile([HK, 512], F32, tag="ps_r")
                nc.tensor.matmul(ps_r[:], sb_so56[:], r8[:], start=True, stop=True)
                wsmC = work.tile([HK, 512], BF, tag="wsmC")
                nc.vector.tensor_mul(wsmC[:], e2[:], ps_r[:])
                # back to token-major: wsm3[p, k, m, h] = w_sm[128m+p, 7h+k]
                ptr = ps_wl.tile([P, 4, HK], BF, tag="w1")
                for j in range(4):
                    nc.tensor.transpose(ptr[:, j, :], wsmC[:, ts(j, P)],
                                        sb_id16[0:HK, 0:HK])
                w_dst = wsm3[:, :, ts(n, 4), :].transpose([0, 2, 3, 1])
                nc.vector.tensor_copy(
                    w_dst, ptr[:].rearrange("p m (h k) -> p m h k", k=K))

            def build_group(mlo, mhi):
                # shifted copies of wsm3 feeding the band scatter
                for i in range(K):
                    d = i - 3
                    kk = 6 - i
                    if d == 0:
                        nc.sync.dma_start(data_tmp[:, i, mlo:mhi, :],
                                          wsm3[:, kk, mlo:mhi, :])
                    elif d < 0:
                        nc.sync.dma_start(data_tmp[-d:P, i, mlo:mhi, :],
                                          wsm3[0:P + d, kk, mlo:mhi, :])
                        lo = max(mlo, 1)
                        if lo < mhi:
                            nc.sync.dma_start(data_tmp[0:-d, i, lo:mhi, :],
                                              wsm3[P + d:P, kk, lo - 1:mhi - 1, :])
                    else:
                        nc.sync.dma_start(data_tmp[0:P - d, i, mlo:mhi, :],
                                          wsm3[d:P, kk, mlo:mhi, :])
                        hi = min(mhi, NT - 1)
                        if mlo < hi:
                            nc.sync.dma_start(data_tmp[P - d:P, i, mlo:hi, :],
                                              wsm3[0:d, kk, mlo + 1:hi + 1, :])
                # permute [p, i, m, h] -> [p, m, (i, h)]
                da4 = data_all[:, mlo:mhi, :].rearrange("p m (i h) -> p m i h", h=H)
                nc.vector.tensor_copy(
                    da4, data_tmp[:, :, mlo:mhi, :].transpose([0, 2, 1, 3]))

            GROUP = min(8, NT)
            n_groups = (NT + GROUP - 1) // GROUP
            for m in range(NT):
                pass1b_tile(m)
            # close 1b PSUM pools; 1c pools take their banks, the conv pools
            # below take the remaining free banks so 1c and conv can overlap
            ctx_exit()
            ps_wl = ctx_enter(tc.tile_pool(name="ps_wl", bufs=1,
                                           space=bass.MemorySpace.PSUM))
            ps_ss = ctx_enter(tc.tile_pool(name="ps_ss", bufs=1,
                                           space=bass.MemorySpace.PSUM))
            for g in range(n_groups):
                for n in range(g * GROUP // 4, min((g + 1) * GROUP, NT) // 4):
                    pass1c_tile(n)
                if g >= 1:
                    build_group((g - 1) * GROUP, g * GROUP)
            build_group((n_groups - 1) * GROUP, NT)

            # ======== pass 2: banded-matmul conv + output matmul ========
            # One wide matmul (N=134) per (h, tile); psum tiles of adjacent
            # time tiles overlap by 3 columns, resolved by DVE edge adds.
            ps_c = ctx_enter(tc.tile_pool(name="ps_c", bufs=2,
                                          space=bass.MemorySpace.PSUM))
            ps_o = ctx_enter(tc.tile_pool(name="ps_o", bufs=1,
                                          space=bass.MemorySpace.PSUM))
            CW = P + 2 * PAD_L  # 134 band columns per tile

            def conv_matmuls(m):
                dt = dtp.tile([P, DT_W], BF, tag="dt")
                nc.gpsimd.local_scatter(dt[:], data_all[:, m, :], sb_idxs[:],
                                        channels=P, num_elems=DT_W, num_idxs=HK)
                if dbg:
                    nc.sync.dma_start(dt_dbg[:, m, :], dt[:])
                # [128, 4, 256] f32 = two PSUM banks; each 134-wide plane pair
                # stays inside a single bank
                pc = ps_c.tile([P, 4, 256], F32, tag="pc")
                pc = pc[:, :, 0:CW]
                for ci in range(4):
                    for hp, pb in ((0, 0), (1, 64)):
                        hh = ci * 2 + hp
                        nc.tensor.matmul(
                            pc[pb:pb + 64, ci, :], xg[:, m, ts(hh, 64)],
                            dt[:, MAIN_W * hh:MAIN_W * hh + CW],
                            start=True, stop=True, skip_group_check=True)
                return pc

            def mm_out(m):
                po = ps_o.tile([P, C], F32, tag="po")
                for q in range(4):
                    nc.tensor.matmul(po[:], conv[:, q, ts(m, P)], sb_woutT[:, q, :],
                                     start=(q == 0), stop=(q == 3))
                out_t = outp.tile([P, C], F32, tag="out_t")
                if with_bias_out:
                    nc.vector.tensor_add(out_t[:], po[:], sb_bout[:])
                else:
                    nc.vector.tensor_copy(out_t[:], po[:])
                nc.sync.dma_start(y_d[ts(m, P), :], out_t[:])

            el_prev = None
            for m in range(NT):
                pc_m = conv_matmuls(m)
                t0 = m * P
                # body of tile m (must precede the left-edge add)
                if with_conv_bias:
                    for ci in range(4):
                        nc.vector.tensor_scalar_add(
                            conv[:, ci, t0:t0 + P], pc_m[:, ci, PAD_L:PAD_L + P],
                            sb_cb4[:, ci:ci + 1])
                else:
                    nc.scalar.copy(conv[:, :, t0:t0 + P],
                                   pc_m[:, :, PAD_L:PAD_L + P])
                if el_prev is not None:
                    # left edge of tile m: slab m-1 rows feeding t0..t0+2
                    dl = conv[:, :, t0:t0 + PAD_L]
                    nc.vector.tensor_add(dl, dl, el_prev[:])
                    # right edge of tile m-1: slab m rows feeding its tail
                    dr = conv[:, :, t0 - PAD_L:t0]
                    nc.vector.tensor_add(dr, dr, pc_m[:, :, 0:PAD_L])
                    mm_out(m - 1)
                if m + 1 < NT:
                    # stage the outgoing right-edge so pc needs one generation
                    el = work.tile([P, 4, PAD_L], F32, tag="el")
                    nc.vector.tensor_copy(el[:], pc_m[:, :, CW - PAD_L:CW])
                    el_prev = el
            mm_out(NT - 1)

            ctx_exit()  # release pass-2 PSUM pools

            if dbg:
                nc.sync.dma_start(xg_dbg[:], xg[:])
                nc.sync.dma_start(xgT_dbg[:], xgT[:])
                nc.sync.dma_start(wsm_dbg[:], wsm3[:])
                nc.sync.dma_start(data_dbg[:], data_all[:])
                nc.sync.dma_start(conv_dbg[:], conv[:])

    nc.compile()
    return nc


def host_inputs(x_b, w_in, b_in, w_wt, b_wt, w_out, b_out, conv_bias,
                with_bias_in, with_bias_wt, with_bias_out, with_conv_bias):
    """Per-core input map from a batch slice + shared weights."""
    def t_pack(w, width):
        # w: [width, C] -> [128, 4, width] bf16 with [p, q, f] = w[f, 128q+p]
        a = np.ascontiguousarray(
            w.T.reshape(4, P, width).transpose(1, 0, 2)).astype(BF16)
        return a

    hk_of = np.arange(HK) // K
    m = {
        "xT": np.ascontiguousarray(np.asarray(x_b, np.float32).T),
        "w_inT": t_pack(w_in, C2),
        "w_wtT": t_pack(w_wt, HK),
        "w_outT": t_pack(w_out, C),
        "idxs": host_scatter_idxs(),
        "ident16": np.eye(P).astype(BF16),
        "sones8": (hk_of[:, None] == np.arange(H)[None, :]).astype(BF16),
        "sones56": (np.arange(H)[:, None] == hk_of[None, :]).astype(BF16),
    }
    if with_bias_in:
        m["b_in"] = np.asarray(b_in, np.float32)
    if with_bias_wt:
        m["b_wt"] = np.asarray(b_wt, np.float32)
    if with_bias_out:
        m["b_out"] = np.asarray(b_out, np.float32)
    if with_conv_bias:
        m["cb4"] = np.ascontiguousarray(
            np.asarray(conv_bias, np.float32).reshape(4, P).T)
    return m


_NC_CACHE = {}


def _get_nc(key):
    if key not in _NC_CACHE:
        _NC_CACHE[key] = build_nc(T, *key)
    return _NC_CACHE[key]


def kernel(x, w_in, b_in, w_wt, b_wt, w_out, b_out, conv_bias, _trace=False):
    x = np.asarray(x)
    flags = (bool(np.any(b_in)), bool(np.any(b_wt)), bool(np.any(b_out)),
             bool(np.any(conv_bias)))
    nc = _get_nc(flags)
    in_maps = [
        host_inputs(x[:, b, :], np.asarray(w_in), b_in, np.asarray(w_wt), b_wt,
                    np.asarray(w_out), b_out, conv_bias, *flags)
        for b in range(B)
    ]
    res = run_bass_kernel_spmd(nc, in_maps, core_ids=list(range(B)),
                               trace=_trace)
    y = np.stack([np.asarray(res.results[b]["y"]) for b in range(B)], axis=1)
    if _trace:
        return y.astype(np.float32), res
    return y.astype(np.float32)


# revision 68
# speedup vs baseline: 1.0164x; 1.0164x over previous
"""Trainium2 Bass kernel for nn_DynamicConvolution.

Reference computation (per batch b, T=4096 timesteps, C=512 channels):
    h  = x @ w_in.T + b_in                    # (T, 2C)
    xg = h[:, :C] * sigmoid(h[:, C:])         # GLU -> (T, C)
    w  = softmax((xg @ w_wt.T + b_wt).reshape(T, H, K), axis=-1)
    out[c, t] = sum_k xg[t+k-3, c] * w[t, h(c), k]    # depthwise dynamic conv
    y  = (out + conv_bias) @ w_out.T + b_out

Sharding: data-parallel over batch B=8 -> one batch element per NeuronCore.
Each core runs an identical program on its slice; no collectives.

Per-core dataflow (all matmuls bf16, fp32 accumulation):
  - x is PE-transposed to xT (C-major) to feed mm1 (contraction over C).
  - mm1 produces h token-major; GLU on ACT+DVE -> xg (token-major, bf16).
  - xg is PE-transposed to xgT for the weight-projection matmul.
  - softmax over K on DVE/ACT -> wsm stored [p, j, m] (token-major).
  - The dynamic conv is computed as a banded matmul per (h, time-tile):
    out_h = xg_slab.T @ D, where D[t', t] = w[h, t'-t+3, t] is a 7-diagonal
    band matrix. D is materialized with one gpsimd local_scatter per time
    tile from a pre-shifted copy of the softmax weights (data_all); the
    per-partition scatter indices are host-precomputed constants.
  - Cross-tile band halo is handled by a second tiny matmul (N=4 columns)
    accumulating into the next tile's PSUM.
  - mm_out contracts C (conv output is C-major already) -> y.
"""

import os
import sys

import numpy as np

for _p in ("/opt/trn_rl_repo", os.path.expanduser("~/.axon_site/_ro/trn_rl_repo")):
    if os.path.isdir(_p) and _p not in sys.path:
        sys.path.insert(0, _p)

import concourse.bacc as bacc
import concourse.bass as bass
import concourse.mybir as mybir
import concourse.tile as tile
from concourse.bass_utils import run_bass_kernel_spmd

try:
    import ml_dtypes

    BF16 = np.dtype(ml_dtypes.bfloat16)
except ImportError:  # pragma: no cover
    BF16 = None

T, B, C = 4096, 8, 512
H, K = 8, 7
PAD_L = K // 2
C2 = 2 * C
HK = H * K  # 56
P = 128

F32 = mybir.dt.float32
BF = mybir.dt.bfloat16
I16 = mybir.dt.int16

# Dt tile layout: per h a 136-wide block holding the 134 band columns of one
# 128-timestep tile (columns j <-> t = t0 + j - 3).
MAIN_W = 136
DT_W = H * MAIN_W  # 1088


def ts(i, size):
    return slice(i * size, (i + 1) * size)


def host_scatter_idxs():
    """Scatter index table: data element (p, i, h) -> column of the Dt tile.

    data[p, i*8+h] = wsm[t0 + p + i - 3, 7h + 6 - i]; its band column is
    j = p + i (column j of block h covers output time t0 + j - 3).
    """
    p = np.arange(P)[:, None, None]
    i = np.arange(K)[None, :, None]
    h = np.arange(H)[None, None, :]
    idx = MAIN_W * h + p + i
    return np.ascontiguousarray(idx.reshape(P, K * H).astype(np.int16))


def build_nc(t_len=T, with_bias_in=False, with_bias_wt=False, with_bias_out=False,
             with_conv_bias=False, dbg=False):
    """Build the single-core Bass program (shared by all 8 cores)."""
    NT = t_len // P  # time tiles of 128
    NT4 = t_len // 512  # time tiles of 512 used by mm1/mm_out

    nc = bacc.Bacc()

    from contextlib import ExitStack
    _psum_stack = ExitStack()

    def ctx_enter(cm):
        return _psum_stack.enter_context(cm)

    def ctx_exit():
        _psum_stack.close()

    x_d = nc.declare_dram_parameter("xT", [C, t_len], F32, isOutput=False)
    w_inT_d = nc.declare_dram_parameter("w_inT", [P, 4, C2], BF, isOutput=False)
    w_wtT_d = nc.declare_dram_parameter("w_wtT", [P, 4, HK], BF, isOutput=False)
    w_outT_d = nc.declare_dram_parameter("w_outT", [P, 4, C], BF, isOutput=False)
    idxs_d = nc.declare_dram_parameter("idxs", [P, HK], I16, isOutput=False)
    ident16_d = nc.declare_dram_parameter("ident16", [P, P], BF, isOutput=False)
    sones8_d = nc.declare_dram_parameter("sones8", [HK, H], BF, isOutput=False)
    sones56_d = nc.declare_dram_parameter("sones56", [H, HK], BF, isOutput=False)
    if with_bias_in:
        b_in_d = nc.declare_dram_parameter("b_in", [C2], F32, isOutput=False)
    if with_bias_wt:
        b_wt_d = nc.declare_dram_parameter("b_wt", [HK], F32, isOutput=False)
    if with_bias_out:
        b_out_d = nc.declare_dram_parameter("b_out", [C], F32, isOutput=False)
    if with_conv_bias:
        cb4_d = nc.declare_dram_parameter("cb4", [P, 4], F32, isOutput=False)
    y_d = nc.declare_dram_parameter("y", [t_len, C], F32, isOutput=True)
    if dbg:
        NTd = t_len // P
        xg_dbg = nc.declare_dram_parameter("xg_dbg", [P, NTd, C], BF, isOutput=True)
        xgT_dbg = nc.declare_dram_parameter("xgT_dbg", [P, 4, t_len], BF, isOutput=True)
        wsm_dbg = nc.declare_dram_parameter("wsm_dbg", [P, K, NTd, H], BF, isOutput=True)
        data_dbg = nc.declare_dram_parameter("data_dbg", [P, NTd, HK], BF, isOutput=True)
        conv_dbg = nc.declare_dram_parameter("conv_dbg", [P, 4, t_len], BF, isOutput=True)
        dt_dbg = nc.declare_dram_parameter("dt_dbg", [P, NTd, DT_W], BF, isOutput=True)

    with tile.TileContext(nc) as tc:
        with (
            tc.tile_pool(name="const", bufs=1) as const,
            tc.tile_pool(name="big", bufs=1) as big,
            tc.tile_pool(name="xin", bufs=3) as xin,
            tc.tile_pool(name="work", bufs=3) as work,
            tc.tile_pool(name="dtp", bufs=3) as dtp,
            tc.tile_pool(name="outp", bufs=3) as outp,
        ):
            # ---- constants ----
            sb_winT = const.tile([P, 4, C2], BF)
            nc.sync.dma_start(sb_winT[:], w_inT_d[:])
            sb_wwtT = const.tile([P, 4, HK], BF)
            nc.sync.dma_start(sb_wwtT[:], w_wtT_d[:])
            sb_woutT = const.tile([P, 4, C], BF)
            nc.sync.dma_start(sb_woutT[:], w_outT_d[:])
            sb_idxs = const.tile([P, HK], I16)
            nc.sync.dma_start(sb_idxs[:], idxs_d[:])
            sb_id16 = const.tile([P, P], BF)
            nc.sync.dma_start(sb_id16[:], ident16_d[:])
            sb_so8 = const.tile([HK, H], BF)
            nc.sync.dma_start(sb_so8[:], sones8_d[:])
            sb_so56 = const.tile([H, HK], BF)
            nc.sync.dma_start(sb_so56[:], sones56_d[:])
            if with_bias_in:
                sb_bin = const.tile([P, C2], F32)
                nc.sync.dma_start(sb_bin[:], b_in_d[None, :].to_broadcast((P, C2)))
            if with_bias_wt:
                sb_bwt = const.tile([HK, 1], F32)
                nc.sync.dma_start(sb_bwt[:], b_wt_d[:, None])
            if with_bias_out:
                sb_bout = const.tile([P, C], F32)
                nc.sync.dma_start(sb_bout[:], b_out_d[None, :].to_broadcast((P, C)))
            if with_conv_bias:
                sb_cb4 = const.tile([P, 4], F32)
                nc.sync.dma_start(sb_cb4[:], cb4_d[:])

            # ---- persistent activations ----
            xT = big.tile([P, 4, t_len], BF)       # [c%128, c//128, t]
            xg = big.tile([P, NT, C], BF)          # [t%128, t//128, c]
            xgT = big.tile([P, 4, t_len], BF)      # [c%128, c//128, t]
            conv = big.tile([P, 4, t_len], BF)     # [c%128, c//128, t]
            wsm3 = big.tile([P, K, NT, H], BF)     # [t%128, k, t//128, h]
            data_tmp = big.tile([P, K, NT, H], BF)
            data_all = big.tile([P, NT, HK], BF)

            # ======== pass 1a: xT load (host pre-transposed), f32->bf16
            # cast by the SWDGE dma ========
            ps_mm1 = ctx_enter(tc.tile_pool(name="ps_mm1", bufs=2,
                                            space=bass.MemorySpace.PSUM))
            ps_tr = ctx_enter(tc.tile_pool(name="ps_tr", bufs=2,
                                           space=bass.MemorySpace.PSUM))
            for m4 in range(NT4):
                for q in range(4):
                    nc.gpsimd.dma_start(xT[:, q, ts(m4, 512)],
                                        x_d[ts(q, P), ts(m4, 512)])

            nc.gpsimd.memset(data_tmp[:], 0.0)

            def pass1b_tile(m):
                ps_a = ps_mm1.tile([P, C], F32, tag="ps_a")
                ps_g = ps_mm1.tile([P, C], F32, tag="ps_g")
                for q in range(4):
                    lhs = xT[:, q, ts(m, P)]
                    nc.tensor.matmul(ps_a[:], lhs, sb_winT[:, q, 0:C],
                                     start=(q == 0), stop=(q == 3))
                    nc.tensor.matmul(ps_g[:], lhs, sb_winT[:, q, C:C2],
                                     start=(q == 0), stop=(q == 3))
                sig = work.tile([P, C], F32, tag="sig")
                if with_bias_in:
                    tmp_g = work.tile([P, C], F32, tag="tmp_g")
                    nc.vector.tensor_add(tmp_g[:], ps_g[:], sb_bin[:, C:C2])
                    nc.scalar.activation(sig[:], tmp_g[:],
                                         mybir.ActivationFunctionType.Sigmoid)
                    tmp_a = work.tile([P, C], F32, tag="tmp_a")
                    nc.vector.tensor_add(tmp_a[:], ps_a[:], sb_bin[:, 0:C])
                    nc.vector.tensor_mul(xg[:, m, :], tmp_a[:], sig[:])
                else:
                    nc.scalar.activation(sig[:], ps_g[:],
                                         mybir.ActivationFunctionType.Sigmoid)
                    nc.vector.tensor_mul(xg[:, m, :], ps_a[:], sig[:])
                # xg -> xgT via PE transpose
                pxgT = ps_tr.tile([P, 4, P], BF, tag="pxgT")
                for q in range(4):
                    nc.tensor.transpose(pxgT[:, q, :], xg[:, m, ts(q, P)], sb_id16[:])
                nc.scalar.copy(xgT[:, :, ts(m, P)], pxgT[:])

            def pass1c_tile(n):
                # dynamic weights + softmax in the C-major [hk, t] domain:
                # exp (logits are bounded, no max-subtract), K-sums and 1/s
                # broadcast via tiny PE matmuls, PE-transpose to token-major
                pw2 = ps_wl.tile([HK, 512], F32, tag="w1")
                for q in range(4):
                    nc.tensor.matmul(pw2[:], sb_wwtT[:, q, :],
                                     xgT[:, q, ts(n, 512)],
                                     start=(q == 0), stop=(q == 3))
                e2 = work.tile([HK, 512], BF, tag="e2")
                if with_bias_wt:
                    nc.scalar.activation(e2[:], pw2[:],
                                         mybir.ActivationFunctionType.Exp,
                                         bias=sb_bwt[:])
                else:
                    nc.scalar.activation(e2[:], pw2[:],
                                         mybir.ActivationFunctionType.Exp)
                ps_s = ps_ss.tile([H, 512], F32, tag="ps_s")
                nc.tensor.matmul(ps_s[:], sb_so8[:], e2[:], start=True, stop=True)
                r8f = work.tile([H, 512], F32, tag="r8f")
                nc.vector.reciprocal_approx_fast(r8f[:], ps_s[:])
                r8 = work.tile([H, 512], BF, tag="r8")
                with nc.allow_low_precision(reason="softmax 1/s in bf16 is fine"):
                    nc.vector.tensor_copy(r8[:], r8f[:])
                ps_r = ps_wl.tile([HK, 512], F32, tag="w1")
                nc.tensor.matmul(ps_r[:], sb_so56[:], r8[:], start=True, stop=True)
                wsmC = work.tile([HK, 512], BF, tag="wsmC")
                nc.vector.tensor_mul(wsmC[:], e2[:], ps_r[:])
                # back to token-major: wsm3[p, k, m, h] = w_sm[128m+p, 7h+k]
                ptr = ps_wl.tile([P, 4, HK], BF, tag="w1")
                for j in range(4):
                    nc.tensor.transpose(ptr[:, j, :], wsmC[:, ts(j, P)],
                                        sb_id16[0:HK, 0:HK])
                w_dst = wsm3[:, :, ts(n, 4), :].transpose([0, 2, 3, 1])
                nc.vector.tensor_copy(
                    w_dst, ptr[:].rearrange("p m (h k) -> p m h k", k=K))

            def build_group(mlo, mhi):
                # shifted copies of wsm3 feeding the band scatter
                for i in range(K):
                    d = i - 3
                    kk = 6 - i
                    if d == 0:
                        nc.sync.dma_start(data_tmp[:, i, mlo:mhi, :],
                                          wsm3[:, kk, mlo:mhi, :])
                    elif d < 0:
                        nc.sync.dma_start(data_tmp[-d:P, i, mlo:mhi, :],
                                          wsm3[0:P + d, kk, mlo:mhi, :])
                        lo = max(mlo, 1)
                        if lo < mhi:
                            nc.sync.dma_start(data_tmp[0:-d, i, lo:mhi, :],
                                              wsm3[P + d:P, kk, lo - 1:mhi - 1, :])
                    else:
                        nc.sync.dma_start(data_tmp[0:P - d, i, mlo:mhi, :],
                                          wsm3[d:P, kk, mlo:mhi, :])
                        hi = min(mhi, NT - 1)
                        if mlo < hi:
                            nc.sync.dma_start(data_tmp[P - d:P, i, mlo:hi, :],
                                              wsm3[0:d, kk, mlo + 1:hi + 1, :])
                # permute [p, i, m, h] -> [p, m, (i, h)]
                da4 = data_all[:, mlo:mhi, :].rearrange("p m (i h) -> p m i h", h=H)
                nc.vector.tensor_copy(
                    da4, data_tmp[:, :, mlo:mhi, :].transpose([0, 2, 1, 3]))

            GROUP = min(8, NT)
            n_groups = (NT + GROUP - 1) // GROUP
            for m in range(NT):
                pass1b_tile(m)
            # close 1b PSUM pools; 1c pools take their banks, the conv pools
            # below take the remaining free banks so 1c and conv can overlap
            ctx_exit()
            ps_wl = ctx_enter(tc.tile_pool(name="ps_wl", bufs=2,
                                           space=bass.MemorySpace.PSUM))
            ps_ss = ctx_enter(tc.tile_pool(name="ps_ss", bufs=1,
                                           space=bass.MemorySpace.PSUM))
            for g in range(n_groups):
                for n in range(g * GROUP // 4, min((g + 1) * GROUP, NT) // 4):
                    pass1c_tile(n)
                if g >= 1:
                    build_group((g - 1) * GROUP, g * GROUP)
            build_group((n_groups - 1) * GROUP, NT)

            # ======== pass 2: banded-matmul conv + output matmul ========
            # One wide matmul (N=134) per (h, tile); psum tiles of adjacent
            # time tiles overlap by 3 columns, resolved by DVE edge adds.
            ps_c = ctx_enter(tc.tile_pool(name="ps_c", bufs=2,
                                          space=bass.MemorySpace.PSUM))
            ps_o = ctx_enter(tc.tile_pool(name="ps_o", bufs=1,
                                          space=bass.MemorySpace.PSUM))
            CW = P + 2 * PAD_L  # 134 band columns per tile

            def conv_matmuls(m):
                dt = dtp.tile([P, DT_W], BF, tag="dt")
                nc.gpsimd.local_scatter(dt[:], data_all[:, m, :], sb_idxs[:],
                                        channels=P, num_elems=DT_W, num_idxs=HK)
                if dbg:
                    nc.sync.dma_start(dt_dbg[:, m, :], dt[:])
                # [128, 4, 256] f32 = two PSUM banks; each 134-wide plane pair
                # stays inside a single bank
                pc = ps_c.tile([P, 4, 256], F32, tag="pc")
                pc = pc[:, :, 0:CW]
                for ci in range(4):
                    for hp, pb in ((0, 0), (1, 64)):
                        hh = ci * 2 + hp
                        nc.tensor.matmul(
                            pc[pb:pb + 64, ci, :], xg[:, m, ts(hh, 64)],
                            dt[:, MAIN_W * hh:MAIN_W * hh + CW],
                            start=True, stop=True, skip_group_check=True)
                return pc

            def mm_out(m):
                po = ps_o.tile([P, C], F32, tag="po")
                for q in range(4):
                    nc.tensor.matmul(po[:], conv[:, q, ts(m, P)], sb_woutT[:, q, :],
                                     start=(q == 0), stop=(q == 3))
                out_t = outp.tile([P, C], F32, tag="out_t")
                if with_bias_out:
                    nc.vector.tensor_add(out_t[:], po[:], sb_bout[:])
                else:
                    nc.vector.tensor_copy(out_t[:], po[:])
                nc.sync.dma_start(y_d[ts(m, P), :], out_t[:])

            el_prev = None
            for m in range(NT):
                pc_m = conv_matmuls(m)
                t0 = m * P
                # body of tile m (must precede the left-edge add)
                if with_conv_bias:
                    for ci in range(4):
                        nc.vector.tensor_scalar_add(
                            conv[:, ci, t0:t0 + P], pc_m[:, ci, PAD_L:PAD_L + P],
                            sb_cb4[:, ci:ci + 1])
                else:
                    nc.scalar.copy(conv[:, :, t0:t0 + P],
                                   pc_m[:, :, PAD_L:PAD_L + P])
                if el_prev is not None:
                    # left edge of tile m: slab m-1 rows feeding t0..t0+2
                    dl = conv[:, :, t0:t0 + PAD_L]
                    nc.vector.tensor_add(dl, dl, el_prev[:])
                    # right edge of tile m-1: slab m rows feeding its tail
                    dr = conv[:, :, t0 - PAD_L:t0]
                    nc.vector.tensor_add(dr, dr, pc_m[:, :, 0:PAD_L])
                    mm_out(m - 1)
                if m + 1 < NT:
                    # stage the outgoing right-edge so pc needs one generation
                    el = work.tile([P, 4, PAD_L], F32, tag="el")
                    nc.vector.tensor_copy(el[:], pc_m[:, :, CW - PAD_L:CW])
                    el_prev = el
            mm_out(NT - 1)

            ctx_exit()  # release pass-2 PSUM pools

            if dbg:
                nc.sync.dma_start(xg_dbg[:], xg[:])
                nc.sync.dma_start(xgT_dbg[:], xgT[:])
                nc.sync.dma_start(wsm_dbg[:], wsm3[:])
                nc.sync.dma_start(data_dbg[:], data_all[:])
                nc.sync.dma_start(conv_dbg[:], conv[:])

    nc.compile()
    return nc


def host_inputs(x_b, w_in, b_in, w_wt, b_wt, w_out, b_out, conv_bias,
                with_bias_in, with_bias_wt, with_bias_out, with_conv_bias):
    """Per-core input map from a batch slice + shared weights."""
    def t_pack(w, width):
        # w: [width, C] -> [128, 4, width] bf16 with [p, q, f] = w[f, 128q+p]
        a = np.ascontiguousarray(
            w.T.reshape(4, P, width).transpose(1, 0, 2)).astype(BF16)
        return a

    hk_of = np.arange(HK) // K
    m = {
        "xT": np.ascontiguousarray(np.asarray(x_b, np.float32).T),
        "w_inT": t_pack(w_in, C2),
        "w_wtT": t_pack(w_wt, HK),
        "w_outT": t_pack(w_out, C),
        "idxs": host_scatter_idxs(),
        "ident16": np.eye(P).astype(BF16),
        "sones8": (hk_of[:, None] == np.arange(H)[None, :]).astype(BF16),
        "sones56": (np.arange(H)[:, None] == hk_of[None, :]).astype(BF16),
    }
    if with_bias_in:
        m["b_in"] = np.asarray(b_in, np.float32)
    if with_bias_wt:
        m["b_wt"] = np.asarray(b_wt, np.float32)
    if with_bias_out:
        m["b_out"] = np.asarray(b_out, np.float32)
    if with_conv_bias:
        m["cb4"] = np.ascontiguousarray(
            np.asarray(conv_bias, np.float32).reshape(4, P).T)
    return m


_NC_CACHE = {}


def _get_nc(key):
    if key not in _NC_CACHE:
        _NC_CACHE[key] = build_nc(T, *key)
    return _NC_CACHE[key]


def kernel(x, w_in, b_in, w_wt, b_wt, w_out, b_out, conv_bias, _trace=False):
    x = np.asarray(x)
    flags = (bool(np.any(b_in)), bool(np.any(b_wt)), bool(np.any(b_out)),
             bool(np.any(conv_bias)))
    nc = _get_nc(flags)
    in_maps = [
        host_inputs(x[:, b, :], np.asarray(w_in), b_in, np.asarray(w_wt), b_wt,
                    np.asarray(w_out), b_out, conv_bias, *flags)
        for b in range(B)
    ]
    res = run_bass_kernel_spmd(nc, in_maps, core_ids=list(range(B)),
                               trace=_trace)
    y = np.stack([np.asarray(res.results[b]["y"]) for b in range(B)], axis=1)
    if _trace:
        return y.astype(np.float32), res
    return y.astype(np.float32)


# revision 69
# speedup vs baseline: 1.1608x; 1.1420x over previous
"""Trainium2 Bass kernel for nn_DynamicConvolution.

Reference computation (per batch b, T=4096 timesteps, C=512 channels):
    h  = x @ w_in.T + b_in                    # (T, 2C)
    xg = h[:, :C] * sigmoid(h[:, C:])         # GLU -> (T, C)
    w  = softmax((xg @ w_wt.T + b_wt).reshape(T, H, K), axis=-1)
    out[c, t] = sum_k xg[t+k-3, c] * w[t, h(c), k]    # depthwise dynamic conv
    y  = (out + conv_bias) @ w_out.T + b_out

Sharding: data-parallel over batch B=8 -> one batch element per NeuronCore.
Each core runs an identical program on its slice; no collectives.

Per-core dataflow (all matmuls bf16, fp32 accumulation):
  - x is PE-transposed to xT (C-major) to feed mm1 (contraction over C).
  - mm1 produces h token-major; GLU on ACT+DVE -> xg (token-major, bf16).
  - xg is PE-transposed to xgT for the weight-projection matmul.
  - softmax over K on DVE/ACT -> wsm stored [p, j, m] (token-major).
  - The dynamic conv is computed as a banded matmul per (h, time-tile):
    out_h = xg_slab.T @ D, where D[t', t] = w[h, t'-t+3, t] is a 7-diagonal
    band matrix. D is materialized with one gpsimd local_scatter per time
    tile from a pre-shifted copy of the softmax weights (data_all); the
    per-partition scatter indices are host-precomputed constants.
  - Cross-tile band halo is handled by a second tiny matmul (N=4 columns)
    accumulating into the next tile's PSUM.
  - mm_out contracts C (conv output is C-major already) -> y.
"""

import os
import sys

import numpy as np

for _p in ("/opt/trn_rl_repo", os.path.expanduser("~/.axon_site/_ro/trn_rl_repo")):
    if os.path.isdir(_p) and _p not in sys.path:
        sys.path.insert(0, _p)

import concourse.bacc as bacc
import concourse.bass as bass
import concourse.mybir as mybir
import concourse.tile as tile
from concourse.bass_utils import run_bass_kernel_spmd

try:
    import ml_dtypes

    BF16 = np.dtype(ml_dtypes.bfloat16)
except ImportError:  # pragma: no cover
    BF16 = None

T, B, C = 4096, 8, 512
H, K = 8, 7
PAD_L = K // 2
C2 = 2 * C
HK = H * K  # 56
P = 128

F32 = mybir.dt.float32
BF = mybir.dt.bfloat16
I16 = mybir.dt.int16

# Dt tile layout: per h a 136-wide block holding the 134 band columns of one
# 128-timestep tile (columns j <-> t = t0 + j - 3).
MAIN_W = 136
DT_W = H * MAIN_W  # 1088


def ts(i, size):
    return slice(i * size, (i + 1) * size)


def host_scatter_idxs():
    """Scatter index table: data element (p, i, h) -> column of the Dt tile.

    data[p, i*8+h] = wsm[t0 + p + i - 3, 7h + 6 - i]; its band column is
    j = p + i (column j of block h covers output time t0 + j - 3).
    """
    p = np.arange(P)[:, None, None]
    i = np.arange(K)[None, :, None]
    h = np.arange(H)[None, None, :]
    idx = MAIN_W * h + p + i
    return np.ascontiguousarray(idx.reshape(P, K * H).astype(np.int16))


def build_nc(t_len=T, with_bias_in=False, with_bias_wt=False, with_bias_out=False,
             with_conv_bias=False, dbg=False):
    """Build the single-core Bass program (shared by all 8 cores)."""
    NT = t_len // P  # time tiles of 128
    NT4 = t_len // 512  # time tiles of 512 used by mm1/mm_out

    nc = bacc.Bacc()

    from contextlib import ExitStack
    _psum_stack = ExitStack()

    def ctx_enter(cm):
        return _psum_stack.enter_context(cm)

    def ctx_exit():
        _psum_stack.close()

    x_d = nc.declare_dram_parameter("xT", [C, t_len], F32, isOutput=False)
    w_inT_d = nc.declare_dram_parameter("w_inT", [P, 4, C2], BF, isOutput=False)
    w_wtT_d = nc.declare_dram_parameter("w_wtT", [P, 4, HK], BF, isOutput=False)
    w_outT_d = nc.declare_dram_parameter("w_outT", [P, 4, C], BF, isOutput=False)
    idxs_d = nc.declare_dram_parameter("idxs", [P, HK], I16, isOutput=False)
    ident16_d = nc.declare_dram_parameter("ident16", [P, P], BF, isOutput=False)
    sones8_d = nc.declare_dram_parameter("sones8", [HK, H], BF, isOutput=False)
    sones56_d = nc.declare_dram_parameter("sones56", [H, HK], BF, isOutput=False)
    if with_bias_in:
        b_in_d = nc.declare_dram_parameter("b_in", [C2], F32, isOutput=False)
    if with_bias_wt:
        b_wt_d = nc.declare_dram_parameter("b_wt", [HK], F32, isOutput=False)
    if with_bias_out:
        b_out_d = nc.declare_dram_parameter("b_out", [C], F32, isOutput=False)
    if with_conv_bias:
        cb4_d = nc.declare_dram_parameter("cb4", [P, 4], F32, isOutput=False)
    y_d = nc.declare_dram_parameter("y", [t_len, C], F32, isOutput=True)
    if dbg:
        NTd = t_len // P
        xg_dbg = nc.declare_dram_parameter("xg_dbg", [P, NTd, C], BF, isOutput=True)
        xgT_dbg = nc.declare_dram_parameter("xgT_dbg", [P, 4, t_len], BF, isOutput=True)
        wsm_dbg = nc.declare_dram_parameter("wsm_dbg", [P, K, NTd, H], BF, isOutput=True)
        data_dbg = nc.declare_dram_parameter("data_dbg", [P, NTd, HK], BF, isOutput=True)
        conv_dbg = nc.declare_dram_parameter("conv_dbg", [P, 4, t_len], BF, isOutput=True)
        dt_dbg = nc.declare_dram_parameter("dt_dbg", [P, NTd, DT_W], BF, isOutput=True)

    with tile.TileContext(nc) as tc:
        with (
            tc.tile_pool(name="const", bufs=1) as const,
            tc.tile_pool(name="big", bufs=1) as big,
            tc.tile_pool(name="xin", bufs=3) as xin,
            tc.tile_pool(name="work", bufs=3) as work,
            tc.tile_pool(name="dtp", bufs=3) as dtp,
            tc.tile_pool(name="outp", bufs=3) as outp,
        ):
            # ---- constants ----
            sb_winT = const.tile([P, 4, C2], BF)
            nc.sync.dma_start(sb_winT[:], w_inT_d[:])
            sb_wwtT = const.tile([P, 4, HK], BF)
            nc.sync.dma_start(sb_wwtT[:], w_wtT_d[:])
            sb_woutT = const.tile([P, 4, C], BF)
            nc.sync.dma_start(sb_woutT[:], w_outT_d[:])
            sb_idxs = const.tile([P, HK], I16)
            nc.sync.dma_start(sb_idxs[:], idxs_d[:])
            sb_id16 = const.tile([P, P], BF)
            nc.sync.dma_start(sb_id16[:], ident16_d[:])
            sb_so8 = const.tile([HK, H], BF)
            nc.sync.dma_start(sb_so8[:], sones8_d[:])
            sb_so56 = const.tile([H, HK], BF)
            nc.sync.dma_start(sb_so56[:], sones56_d[:])
            if with_bias_in:
                sb_bin = const.tile([P, C2], F32)
                nc.sync.dma_start(sb_bin[:], b_in_d[None, :].to_broadcast((P, C2)))
            if with_bias_wt:
                sb_bwt = const.tile([HK, 1], F32)
                nc.sync.dma_start(sb_bwt[:], b_wt_d[:, None])
            if with_bias_out:
                sb_bout = const.tile([P, C], F32)
                nc.sync.dma_start(sb_bout[:], b_out_d[None, :].to_broadcast((P, C)))
            if with_conv_bias:
                sb_cb4 = const.tile([P, 4], F32)
                nc.sync.dma_start(sb_cb4[:], cb4_d[:])

            # ---- persistent activations ----
            xT = big.tile([P, 4, t_len], BF)       # [c%128, c//128, t]
            xg = big.tile([P, NT, C], BF)          # [t%128, t//128, c]
            xgT = big.tile([P, 4, t_len], BF)      # [c%128, c//128, t]
            conv = big.tile([P, 4, t_len], BF)     # [c%128, c//128, t]
            wsm3 = big.tile([P, K, NT, H], BF)     # [t%128, k, t//128, h]
            data_tmp = big.tile([P, K, NT, H], BF)
            data_all = big.tile([P, NT, HK], BF)

            # ======== pass 1a: xT load (host pre-transposed), f32->bf16
            # cast by the SWDGE dma ========
            ps_mm1 = ctx_enter(tc.tile_pool(name="ps_mm1", bufs=2,
                                            space=bass.MemorySpace.PSUM))
            ps_tr = ctx_enter(tc.tile_pool(name="ps_tr", bufs=2,
                                           space=bass.MemorySpace.PSUM))
            for m4 in range(NT4):
                for q in range(4):
                    nc.gpsimd.dma_start(xT[:, q, ts(m4, 512)],
                                        x_d[ts(q, P), ts(m4, 512)])

            nc.gpsimd.memset(data_tmp[:], 0.0)

            def pass1b_tile(m):
                ps_a = ps_mm1.tile([P, C], F32, tag="ps_a")
                ps_g = ps_mm1.tile([P, C], F32, tag="ps_g")
                for q in range(4):
                    lhs = xT[:, q, ts(m, P)]
                    nc.tensor.matmul(ps_a[:], lhs, sb_winT[:, q, 0:C],
                                     start=(q == 0), stop=(q == 3))
                    nc.tensor.matmul(ps_g[:], lhs, sb_winT[:, q, C:C2],
                                     start=(q == 0), stop=(q == 3))
                sig = work.tile([P, C], F32, tag="sig")
                if with_bias_in:
                    tmp_g = work.tile([P, C], F32, tag="tmp_g")
                    nc.vector.tensor_add(tmp_g[:], ps_g[:], sb_bin[:, C:C2])
                    nc.scalar.activation(sig[:], tmp_g[:],
                                         mybir.ActivationFunctionType.Sigmoid)
                    tmp_a = work.tile([P, C], F32, tag="tmp_a")
                    nc.vector.tensor_add(tmp_a[:], ps_a[:], sb_bin[:, 0:C])
                    nc.vector.tensor_mul(xg[:, m, :], tmp_a[:], sig[:])
                else:
                    nc.scalar.activation(sig[:], ps_g[:],
                                         mybir.ActivationFunctionType.Sigmoid)
                    nc.vector.tensor_mul(xg[:, m, :], ps_a[:], sig[:])
                # xg -> xgT via PE transpose
                pxgT = ps_tr.tile([P, 4, P], BF, tag="pxgT")
                for q in range(4):
                    nc.tensor.transpose(pxgT[:, q, :], xg[:, m, ts(q, P)], sb_id16[:])
                nc.scalar.copy(xgT[:, :, ts(m, P)], pxgT[:])

            def pass1c_tile(n):
                # dynamic weights + softmax in the C-major [hk, t] domain:
                # exp (logits are bounded, no max-subtract), K-sums and 1/s
                # broadcast via tiny PE matmuls, PE-transpose to token-major
                pw2 = ps_wl.tile([HK, 512], F32, tag="pw2")
                for q in range(4):
                    nc.tensor.matmul(pw2[:], sb_wwtT[:, q, :],
                                     xgT[:, q, ts(n, 512)],
                                     start=(q == 0), stop=(q == 3))
                e2 = work.tile([HK, 512], BF, tag="e2")
                if with_bias_wt:
                    nc.scalar.activation(e2[:], pw2[:],
                                         mybir.ActivationFunctionType.Exp,
                                         bias=sb_bwt[:])
                else:
                    nc.scalar.activation(e2[:], pw2[:],
                                         mybir.ActivationFunctionType.Exp)
                ps_s = ps_ss.tile([H, 512], F32, tag="ps_s")
                nc.tensor.matmul(ps_s[:], sb_so8[:], e2[:], start=True, stop=True)
                r8f = work.tile([H, 512], F32, tag="r8f")
                nc.vector.reciprocal_approx_fast(r8f[:], ps_s[:])
                r8 = work.tile([H, 512], BF, tag="r8")
                with nc.allow_low_precision(reason="softmax 1/s in bf16 is fine"):
                    nc.vector.tensor_copy(r8[:], r8f[:])
                ps_r = ps_ss.tile([HK, 512], F32, tag="ps_r")
                nc.tensor.matmul(ps_r[:], sb_so56[:], r8[:], start=True, stop=True)
                wsmC = work.tile([HK, 512], BF, tag="wsmC")
                nc.vector.tensor_mul(wsmC[:], e2[:], ps_r[:])
                # back to token-major: wsm3[p, k, m, h] = w_sm[128m+p, 7h+k]
                ptr = ps_wtr.tile([P, 4, HK], BF, tag="ptr")
                for j in range(4):
                    nc.tensor.transpose(ptr[:, j, :], wsmC[:, ts(j, P)],
                                        sb_id16[0:HK, 0:HK])
                w_dst = wsm3[:, :, ts(n, 4), :].transpose([0, 2, 3, 1])
                nc.vector.tensor_copy(
                    w_dst, ptr[:].rearrange("p m (h k) -> p m h k", k=K))

            def build_group(mlo, mhi):
                # shifted copies of wsm3 feeding the band scatter
                for i in range(K):
                    d = i - 3
                    kk = 6 - i
                    if d == 0:
                        nc.sync.dma_start(data_tmp[:, i, mlo:mhi, :],
                                          wsm3[:, kk, mlo:mhi, :])
                    elif d < 0:
                        nc.sync.dma_start(data_tmp[-d:P, i, mlo:mhi, :],
                                          wsm3[0:P + d, kk, mlo:mhi, :])
                        lo = max(mlo, 1)
                        if lo < mhi:
                            nc.sync.dma_start(data_tmp[0:-d, i, lo:mhi, :],
                                              wsm3[P + d:P, kk, lo - 1:mhi - 1, :])
                    else:
                        nc.sync.dma_start(data_tmp[0:P - d, i, mlo:mhi, :],
                                          wsm3[d:P, kk, mlo:mhi, :])
                        hi = min(mhi, NT - 1)
                        if mlo < hi:
                            nc.sync.dma_start(data_tmp[P - d:P, i, mlo:hi, :],
                                              wsm3[0:d, kk, mlo + 1:hi + 1, :])
                # permute [p, i, m, h] -> [p, m, (i, h)]
                da4 = data_all[:, mlo:mhi, :].rearrange("p m (i h) -> p m i h", h=H)
                nc.vector.tensor_copy(
                    da4, data_tmp[:, :, mlo:mhi, :].transpose([0, 2, 1, 3]))

            GROUP = min(8, NT)
            n_groups = (NT + GROUP - 1) // GROUP
            for m in range(NT):
                pass1b_tile(m)
            # close 1b PSUM pools; 1c pools take their banks, the conv pools
            # below take the remaining free banks so 1c and conv can overlap
            ctx_exit()
            ps_wl = ctx_enter(tc.tile_pool(name="ps_wl", bufs=2,
                                           space=bass.MemorySpace.PSUM))
            ps_ss = ctx_enter(tc.tile_pool(name="ps_ss", bufs=2,
                                           space=bass.MemorySpace.PSUM))
            ps_wtr = ctx_enter(tc.tile_pool(name="ps_wtr", bufs=2,
                                            space=bass.MemorySpace.PSUM))
            for g in range(n_groups):
                for n in range(g * GROUP // 4, min((g + 1) * GROUP, NT) // 4):
                    pass1c_tile(n)
                if g >= 1:
                    build_group((g - 1) * GROUP, g * GROUP)
            build_group((n_groups - 1) * GROUP, NT)

            # ======== pass 2: banded-matmul conv + output matmul ========
            # One wide matmul (N=134) per (h, tile); psum tiles of adjacent
            # time tiles overlap by 3 columns, resolved by DVE edge adds.
            ctx_exit()  # release pass-1 PSUM pools
            ps_c = ctx_enter(tc.tile_pool(name="ps_c", bufs=3,
                                          space=bass.MemorySpace.PSUM))
            ps_o = ctx_enter(tc.tile_pool(name="ps_o", bufs=2,
                                          space=bass.MemorySpace.PSUM))
            CW = P + 2 * PAD_L  # 134 band columns per tile

            def conv_matmuls(m):
                dt = dtp.tile([P, DT_W], BF, tag="dt")
                nc.gpsimd.local_scatter(dt[:], data_all[:, m, :], sb_idxs[:],
                                        channels=P, num_elems=DT_W, num_idxs=HK)
                if dbg:
                    nc.sync.dma_start(dt_dbg[:, m, :], dt[:])
                # [128, 4, 256] f32 = two PSUM banks; each 134-wide plane pair
                # stays inside a single bank
                pc = ps_c.tile([P, 4, 256], F32, tag="pc")
                pc = pc[:, :, 0:CW]
                for ci in range(4):
                    for hp, pb in ((0, 0), (1, 64)):
                        hh = ci * 2 + hp
                        nc.tensor.matmul(
                            pc[pb:pb + 64, ci, :], xg[:, m, ts(hh, 64)],
                            dt[:, MAIN_W * hh:MAIN_W * hh + CW],
                            start=True, stop=True, skip_group_check=True)
                return pc

            def mm_out(m):
                po = ps_o.tile([P, C], F32, tag="po")
                for q in range(4):
                    nc.tensor.matmul(po[:], conv[:, q, ts(m, P)], sb_woutT[:, q, :],
                                     start=(q == 0), stop=(q == 3))
                out_t = outp.tile([P, C], F32, tag="out_t")
                if with_bias_out:
                    nc.vector.tensor_add(out_t[:], po[:], sb_bout[:])
                else:
                    nc.vector.tensor_copy(out_t[:], po[:])
                nc.sync.dma_start(y_d[ts(m, P), :], out_t[:])

            el_prev = None
            for m in range(NT):
                pc_m = conv_matmuls(m)
                t0 = m * P
                # body of tile m (must precede the left-edge add)
                if with_conv_bias:
                    for ci in range(4):
                        nc.vector.tensor_scalar_add(
                            conv[:, ci, t0:t0 + P], pc_m[:, ci, PAD_L:PAD_L + P],
                            sb_cb4[:, ci:ci + 1])
                else:
                    nc.scalar.copy(conv[:, :, t0:t0 + P],
                                   pc_m[:, :, PAD_L:PAD_L + P])
                if el_prev is not None:
                    # left edge of tile m: slab m-1 rows feeding t0..t0+2
                    dl = conv[:, :, t0:t0 + PAD_L]
                    nc.vector.tensor_add(dl, dl, el_prev[:])
                    # right edge of tile m-1: slab m rows feeding its tail
                    dr = conv[:, :, t0 - PAD_L:t0]
                    nc.vector.tensor_add(dr, dr, pc_m[:, :, 0:PAD_L])
                    mm_out(m - 1)
                if m + 1 < NT:
                    # stage the outgoing right-edge so pc needs one generation
                    el = work.tile([P, 4, PAD_L], F32, tag="el")
                    nc.vector.tensor_copy(el[:], pc_m[:, :, CW - PAD_L:CW])
                    el_prev = el
            mm_out(NT - 1)

            ctx_exit()  # release pass-2 PSUM pools

            if dbg:
                nc.sync.dma_start(xg_dbg[:], xg[:])
                nc.sync.dma_start(xgT_dbg[:], xgT[:])
                nc.sync.dma_start(wsm_dbg[:], wsm3[:])
                nc.sync.dma_start(data_dbg[:], data_all[:])
                nc.sync.dma_start(conv_dbg[:], conv[:])

    nc.compile()
    return nc


def host_inputs(x_b, w_in, b_in, w_wt, b_wt, w_out, b_out, conv_bias,
                with_bias_in, with_bias_wt, with_bias_out, with_conv_bias):
    """Per-core input map from a batch slice + shared weights."""
    def t_pack(w, width):
        # w: [width, C] -> [128, 4, width] bf16 with [p, q, f] = w[f, 128q+p]
        a = np.ascontiguousarray(
            w.T.reshape(4, P, width).transpose(1, 0, 2)).astype(BF16)
        return a

    hk_of = np.arange(HK) // K
    m = {
        "xT": np.ascontiguousarray(np.asarray(x_b, np.float32).T),
        "w_inT": t_pack(w_in, C2),
        "w_wtT": t_pack(w_wt, HK),
        "w_outT": t_pack(w_out, C),
        "idxs": host_scatter_idxs(),
        "ident16": np.eye(P).astype(BF16),
        "sones8": (hk_of[:, None] == np.arange(H)[None, :]).astype(BF16),
        "sones56": (np.arange(H)[:, None] == hk_of[None, :]).astype(BF16),
    }
    if with_bias_in:
        m["b_in"] = np.asarray(b_in, np.float32)
    if with_bias_wt:
        m["b_wt"] = np.asarray(b_wt, np.float32)
    if with_bias_out:
        m["b_out"] = np.asarray(b_out, np.float32)
    if with_conv_bias:
        m["cb4"] = np.ascontiguousarray(
            np.asarray(conv_bias, np.float32).reshape(4, P).T)
    return m


_NC_CACHE = {}


def _get_nc(key):
    if key not in _NC_CACHE:
        _NC_CACHE[key] = build_nc(T, *key)
    return _NC_CACHE[key]


def kernel(x, w_in, b_in, w_wt, b_wt, w_out, b_out, conv_bias, _trace=False):
    x = np.asarray(x)
    flags = (bool(np.any(b_in)), bool(np.any(b_wt)), bool(np.any(b_out)),
             bool(np.any(conv_bias)))
    nc = _get_nc(flags)
    in_maps = [
        host_inputs(x[:, b, :], np.asarray(w_in), b_in, np.asarray(w_wt), b_wt,
                    np.asarray(w_out), b_out, conv_bias, *flags)
        for b in range(B)
    ]
    res = run_bass_kernel_spmd(nc, in_maps, core_ids=list(range(B)),
                               trace=_trace)
    y = np.stack([np.asarray(res.results[b]["y"]) for b in range(B)], axis=1)
    if _trace:
        return y.astype(np.float32), res
    return y.astype(np.float32)
